# revision 37
# baseline (speedup 1.0000x reference)
"""Multi-head self-attention on Trainium2, 8-core SPMD.

Problem: x[2,2048,1024] -> torch-style MHSA (16 heads, head_dim 64) -> [2,2048,1024]

Sharding (data + tensor parallel): 8 cores = 2 batches x 4 head-groups.
Each core handles one batch and 4 heads: it computes Q/K/V projections for
its 256 channels, attention for its 4 heads, and the out-projection with its
256 rows of Wo, producing a partial [S, E] output. The host sums the 4
head-group partials per batch and adds the output bias.

Kernel design (v3 — fused schedule, fp8 DoubleRow PV, 3-engine softmax):
  * Single fused pipeline instead of proj-then-attention phases: PE emits
    K-projection, Q(block0), half of V, then runs attention; the remaining
    V tiles, later Q blocks and the (one-block-deferred) out-projections
    are interleaved between attention steps as PE filler, so exp latency
    and drain chains never idle the PE.
  * Scores in bf16, transposed orientation (lhsT=K^T, rhs=Q^T, contraction
    dh=64 on PE rows 0-63/64-127 per head of a pair), one [128,QB] psum
    bank per (key-tile, head); 1/sqrt(dh) folded into Wq/bq on the host.
  * PV contraction in fp8e4m3 with the DoubleRow perf mode: each matmul
    contracts TWO key tiles (planes at a fixed free-dim stride on the same
    partitions) at 0.5 cycles/column.
      - vhat ([V_h | ones] per key tile) stored fp8, scaled x8 (folded into
        Wv/bv on the host; cancels exactly in the softmax ratio since the
        ones column is 8 as well).
      - attn probs stored fp8: ACT exp writes float8e4 directly; a subset
        of tiles is instead exp'd on DVE or GPSIMD via a Schraudolph fused
        multiply-add whose uint8 output bits ARE the fp8e4m3 value
        (i8 = round(s*8/ln2 + 55.62), ~1.9% rms) — softmax runs on three
        engines in parallel.
  * Softmax normalization deferred: PV row 64 (ones) is the denominator;
    reciprocal (bf16) partition-broadcast via a DRAM bounce, multiplied
    into the copied-out rows. Diagonal scaling commutes with out-proj.
  * Out-projection contracts over 128 channels in 2 accumulation steps
    over pair-stacked outTs[128, pair, S]: even head written in place by
    DVE, odd head staged and hopped to partitions 64-127 by a small
    sbuf->sbuf DMA. Out psum is drained by GPSIMD copies.
"""

from contextlib import ExitStack

import numpy as np

import concourse.bacc as bacc
import concourse.mybir as mybir
import concourse.tile as tile

P = 128
DH = 64  # head dim
F32 = mybir.dt.float32
BF = mybir.dt.bfloat16
F8 = mybir.dt.float8e4
U8 = mybir.dt.uint8

# full-size problem constants
FULL_B = 2
FULL_S = 2048
FULL_E = 1024
FULL_H = 16
HPC = 4  # heads per core
N_CORES = 8

V_SCALE = 8.0  # vhat fp8 scale, folded into Wv/bv on host; cancels in softmax
SCHRAUD_A = 8.0 / float(np.log(2.0))
SCHRAUD_B = 55.62


def exp_engine(kt, h):
    """Engine for the exp of (key tile kt, head h of the pair): ACT 9/16
    (exact, fp8 out), DVE (Schraudolph) 7/16. GPSIMD cannot read PSUM, so
    only these two; the ratio balances ACT vs DVE total busy time."""
    return "dve" if (2 * kt + h) % 16 in (1, 3, 5, 8, 10, 12, 14) else "act"


def build_nc(S=FULL_S, E=FULL_E, hpc=HPC, reps=1, exp_engine=exp_engine,
             dma_engine="sync"):
    """Build the single-core Bass program (same program on all 8 cores)."""
    assert S % P == 0 and E % P == 0 and hpc % 2 == 0
    HD = hpc * DH            # projected channels per core (256)
    NPAIR = hpc // 2         # head pairs (2)
    EK = E // P              # contraction tiles for projections (8)
    SK = S // P              # key tiles for attention (16)
    SKP = SK // 2            # key tile pairs for the DoubleRow PV (8)
    QB = min(512, S)         # attention query block / proj psum width
    NQB = S // QB
    VN = DH + 1              # V columns per head incl. ones column (65)
    EMW = min(512, E)        # out-projection matmul moving width
    NEB = E // EMW

    nc = bacc.Bacc(trn_type="TRN2", target_bir_lowering=False, debug=False)
    dmae = lambda: getattr(nc, dma_engine)

    xT = nc.declare_dram_parameter("xT", [E, S], BF, isOutput=False)
    wq = nc.declare_dram_parameter("wq", [E, HD], BF, isOutput=False)
    wk = nc.declare_dram_parameter("wk", [E, HD], BF, isOutput=False)
    wv = nc.declare_dram_parameter("wv", [E, HD], BF, isOutput=False)
    wo = nc.declare_dram_parameter("wo", [HD, E], BF, isOutput=False)
    bq = nc.declare_dram_parameter("bq", [P, NPAIR], F32, isOutput=False)
    bk = nc.declare_dram_parameter("bk", [P, NPAIR], F32, isOutput=False)
    bvb = nc.declare_dram_parameter("bvb", [P, HD], F32, isOutput=False)
    out = nc.declare_dram_parameter("out", [S, E], F32, isOutput=True)
    rcp_dram = nc.dram_tensor("rcp_scratch", [hpc, S], BF)

    Exp = mybir.ActivationFunctionType.Exp
    Add = mybir.AluOpType.add
    Mult = mybir.AluOpType.mult
    DR = mybir.MatmulPerfMode.DoubleRow

    with ExitStack() as ctx:
        tc = ctx.enter_context(tile.TileContext(nc))
        for _rep in range(reps):
            rctx = ctx.enter_context(ExitStack())
            const = rctx.enter_context(tc.tile_pool(name="const", bufs=1))
            proj = rctx.enter_context(tc.tile_pool(name="proj", bufs=1))
            xw = rctx.enter_context(tc.tile_pool(name="xw", bufs=1))
            big_ps = rctx.enter_context(
                tc.tile_pool(name="big_ps", bufs=2, space="PSUM"))
            sc_ps = rctx.enter_context(
                tc.tile_pool(name="sc_ps", bufs=4, space="PSUM"))
            pv_ps = rctx.enter_context(
                tc.tile_pool(name="pv_ps", bufs=2, space="PSUM"))
            at_pool = rctx.enter_context(tc.tile_pool(name="at", bufs=10))
            stg_pool = rctx.enter_context(tc.tile_pool(name="stg", bufs=3))
            rb_pool = rctx.enter_context(tc.tile_pool(name="rb", bufs=2))
            rcp_pool = rctx.enter_context(tc.tile_pool(name="rcp", bufs=4))
            ob_pool = rctx.enter_context(tc.tile_pool(name="ob", bufs=3))

            bq_sb = const.tile([P, NPAIR], F32)
            bk_sb = const.tile([P, NPAIR], F32)
            bv_sb = const.tile([P, HD], F32)
            wo_sb = const.tile([P, NPAIR, E], BF)
            # fp8 ones lhsT for the denominator DoubleRow matmuls (all 64
            # output rows identical; only row 0 is read)
            ones8 = const.tile([P, 2, DH], F8)

            # persistent activation tensors
            qt_sb = proj.tile([P, NPAIR, S], BF)   # Q^T (head pair pr on
            kt_sb = proj.tile([P, NPAIR, S], BF)   # partitions 64pr..), K^T
            vhat = proj.tile([P, SK, hpc * DH], F8)  # 8*V per key tile
            outTs = proj.tile([P, NPAIR, S], BF)   # pair-stacked attn out^T

            xt = xw.tile([P, EK, S], BF)
            wq_sb = xw.tile([P, EK, HD], BF)
            wk_sb = xw.tile([P, EK, HD], BF)
            wv_sb = xw.tile([P, EK, HD], BF)

            # DMA order = need order: K path first, then Q, V, out-proj
            xT_t = xT.rearrange("(kt p) s -> kt p s", p=P)
            dmae().dma_start(out=bk_sb[:], in_=bk[:, :])
            dmae().dma_start(out=bq_sb[:], in_=bq[:, :])
            dmae().dma_start(
                out=wk_sb[:], in_=wk.rearrange("(kt p) n -> p kt n", p=P))
            for kt in range(EK):
                dmae().dma_start(out=xt[:, kt, :], in_=xT_t[kt])
            dmae().dma_start(
                out=wq_sb[:], in_=wq.rearrange("(kt p) n -> p kt n", p=P))
            dmae().dma_start(
                out=wv_sb[:], in_=wv.rearrange("(kt p) n -> p kt n", p=P))
            dmae().dma_start(out=bv_sb[:], in_=bvb[:, :])
            dmae().dma_start(
                out=wo_sb[:], in_=wo.rearrange("(pr p) e -> p pr e", p=P))

            nc.vector.memset(ones8[:], 1.0)

            # ---------- building blocks ----------
            def qk_proj_tile(w_sb, b_sb, dst, pr, nb, pool=None):
                """dst[:, pr, nb*QB:(nb+1)*QB] = W_pair^T x xT + bias."""
                ssl = slice(nb * QB, (nb + 1) * QB)
                pool = pool or big_ps
                ps = pool.tile([P, QB], F32, tag="sc" if pool is sc_ps else "big")
                for kt in range(EK):
                    nc.tensor.matmul(
                        ps[:],
                        lhsT=w_sb[:, kt, pr * P:(pr + 1) * P],
                        rhs=xt[:, kt, ssl],
                        start=(kt == 0),
                        stop=(kt == EK - 1),
                    )
                nc.vector.tensor_scalar(
                    out=dst[:, pr, ssl], in0=ps[:],
                    scalar1=b_sb[:, pr:pr + 1], scalar2=None, op0=Add)

            def v_proj_tile(st, pool=None):
                """vhat[:, st, :] = 8*(xT_st^T x Wv + bv)."""
                pool = pool or big_ps
                ps = pool.tile([P, QB], F32, tag="sc" if pool is sc_ps else "big")
                for kt in range(EK):
                    nc.tensor.matmul(
                        ps[:, 0:HD],
                        lhsT=xt[:, kt, st * P:(st + 1) * P],
                        rhs=wv_sb[:, kt, :],
                        start=(kt == 0),
                        stop=(kt == EK - 1),
                    )
                nc.vector.tensor_add(
                    out=vhat[:, st, :],
                    in0=ps[:, 0:HD],
                    in1=bv_sb[:],
                )

            def op_tile(qb, m, nb):
                """One out-projection psum tile: out[msl, esl]. The psum
                drain alternates ACT/DVE (GPSIMD cannot read PSUM)."""
                msl = slice(qb * QB + m * P, qb * QB + (m + 1) * P)
                esl = slice(nb * EMW, (nb + 1) * EMW)
                ps = big_ps.tile([P, EMW], F32, tag="big")
                for pr in range(NPAIR):
                    nc.tensor.matmul(
                        ps[:],
                        lhsT=outTs[:, pr, msl],
                        rhs=wo_sb[:, pr, esl],
                        start=(pr == 0),
                        stop=(pr == NPAIR - 1),
                    )
                ob = ob_pool.tile([P, EMW], F32, tag="ob")
                nc.scalar.copy(out=ob[:], in_=ps[:])
                dmae().dma_start(out=out[msl, esl], in_=ob[:])

            # ---------- pre-attention: K, Q0, V st0-7 ----------
            for pr in range(NPAIR):
                for nb in range(NQB):
                    qk_proj_tile(wk_sb, bk_sb, kt_sb, pr, nb, pool=sc_ps)
            for pr in range(NPAIR):
                qk_proj_tile(wq_sb, bq_sb, qt_sb, pr, 0, pool=sc_ps)
            for st in range(SK // 2):
                v_proj_tile(st, pool=sc_ps)

            # ---------- filler schedule: PE work interleaved into attention
            # (each unit ~0.9-2us); consumed at fixed slots inside a block
            fillers = {
                (0, 0): [lambda s=s: (v_proj_tile(2 * s + 8),
                                      v_proj_tile(2 * s + 9))
                         for s in range(4)],
                (0, 1): [lambda: qk_proj_tile(wq_sb, bq_sb, qt_sb, 0, 1),
                         lambda: qk_proj_tile(wq_sb, bq_sb, qt_sb, 1, 1)],
                (1, 0): [lambda: qk_proj_tile(wq_sb, bq_sb, qt_sb, 0, 2),
                         lambda: qk_proj_tile(wq_sb, bq_sb, qt_sb, 1, 2)],
                (1, 1): [lambda m=m: (op_tile(0, m, 0), op_tile(0, m, 1))
                         for m in range(4)],
                (2, 0): [lambda: qk_proj_tile(wq_sb, bq_sb, qt_sb, 0, 3),
                         lambda: qk_proj_tile(wq_sb, bq_sb, qt_sb, 1, 3)],
                (2, 1): [lambda m=m: (op_tile(1, m, 0), op_tile(1, m, 1))
                         for m in range(4)],
                (3, 0): [],
                (3, 1): [lambda m=m: (op_tile(2, m, 0), op_tile(2, m, 1))
                         for m in range(4)],
            }
            SLOT_TPS = (1, 3, 5, 7)

            # ---------- attention ----------
            for qb in range(NQB):
                qsl = slice(qb * QB, (qb + 1) * QB)
                for pr in range(NPAIR):
                    h0, h1 = 2 * pr, 2 * pr + 1
                    units = list(fillers.get((qb, pr), ()))
                    pv0 = pv_ps.tile([DH, QB], F32, tag="pv")
                    pv1 = pv_ps.tile([DH, QB], F32, tag="pv")
                    at4s = []
                    for tp in range(SKP):
                        # at4 [P, kt-plane, head, QB] holds both key tiles of
                        # the pair for both heads; the PV DoubleRow matmul
                        # contracts the two kt planes (strided dim1 AP)
                        at4 = at_pool.tile([P, 2, 2, QB], F8, tag="at")
                        at4s.append(at4)
                        for i, kt in enumerate((2 * tp, 2 * tp + 1)):
                            ksl = slice(kt * P, (kt + 1) * P)
                            for h in range(2):
                                hsl = slice(h * DH, (h + 1) * DH)
                                sc = sc_ps.tile([P, QB], F32, tag="sc")
                                nc.tensor.matmul(
                                    sc[:],
                                    lhsT=kt_sb[hsl, pr, ksl],
                                    rhs=qt_sb[hsl, pr, qsl],
                                    start=True, stop=True,
                                    tile_position=(h * DH, 0),
                                )
                                if exp_engine(kt, h) == "act":
                                    nc.scalar.activation(
                                        out=at4[:, i, h, :], in_=sc[:],
                                        func=Exp)
                                else:
                                    # Schraudolph: uint8 bits = fp8e4m3
                                    nc.vector.tensor_scalar(
                                        out=at4[:, i, h, :].bitcast(U8),
                                        in0=sc[:],
                                        scalar1=SCHRAUD_A, scalar2=SCHRAUD_B,
                                        op0=Mult, op1=Add,
                                    )
                        if tp in SLOT_TPS and units:
                            units.pop(0)()
                        for h, pv in ((h0, pv0), (h1, pv1)):
                            nc.tensor.matmul(
                                pv[:, :],
                                lhsT=vhat[:, 2 * tp:2 * tp + 2,
                                          h * DH:(h + 1) * DH],
                                rhs=at4[:, :, h % 2, :],
                                start=(tp == 0),
                                stop=(tp == SKP - 1),
                                perf_mode=DR,
                            )
                    while units:
                        units.pop(0)()
                    # copies first: free the pv psum banks immediately
                    dst0 = outTs[0:DH, pr, qsl]
                    nc.vector.tensor_copy(out=dst0, in_=pv0[:])
                    stg = stg_pool.tile([DH, QB], BF, tag="stg")
                    nc.vector.tensor_copy(out=stg[:], in_=pv1[:])
                    # denominator chains (ones lhsT; rows identical, row 0
                    # read); den tiles ride the sc pool's 1-bank slots
                    dens = []
                    for h in (h0, h1):
                        den = sc_ps.tile([DH, QB], F32, tag="sc")
                        dens.append(den)
                        for tp in range(SKP):
                            nc.tensor.matmul(
                                den[:, :],
                                lhsT=ones8[:],
                                rhs=at4s[tp][:, :, h % 2, :],
                                start=(tp == 0),
                                stop=(tp == SKP - 1),
                                perf_mode=DR,
                            )
                    for h, dsl, den in ((h0, dst0, dens[0]),
                                        (h1, stg[:, :], dens[1])):
                        odd = h % 2
                        rcp = rcp_pool.tile([1, QB], BF, tag="rcp")
                        with nc.allow_low_precision(
                                reason="softmax denom reciprocal in bf16"):
                            nc.vector.reciprocal(
                                out=rcp[:], in_=den[0:1, :])
                        # partition-broadcast needs a DRAM source
                        dmae().dma_start(
                            out=rcp_dram[h:h + 1, qsl], in_=rcp[:])
                        rb = rb_pool.tile([DH, QB], BF, tag="rb")
                        dmae().dma_start(
                            out=rb[:],
                            in_=rcp_dram[h:h + 1, qsl].to_broadcast((DH, QB)),
                        )
                        nc.vector.tensor_mul(out=dsl, in0=dsl, in1=rb[:])
                        if odd:
                            # hop the odd head to partitions 64-127
                            dmae().dma_start(
                                out=outTs[DH:P, pr, qsl], in_=dsl)

            # tail: last block's out-projection
            for m in range(QB // P):
                for nb in range(NEB):
                    op_tile(NQB - 1, m, nb)

            rctx.close()

    nc.compile()
    return nc


def make_in_maps(x, Wq, bq, Wk, bk, Wv, bv, Wo, hpc=HPC, n_cores=N_CORES):
    """Host-side sharding: per-core input dict list."""
    import ml_dtypes
    bf16 = ml_dtypes.bfloat16
    x = np.asarray(x, dtype=np.float32)
    B = x.shape[0]
    groups = n_cores // B
    HD = hpc * DH
    scale = 1.0 / np.sqrt(np.float32(DH))
    in_maps = []
    for c in range(n_cores):
        b, g = divmod(c, groups)
        hs = slice(g * HD, (g + 1) * HD)
        bq_s = (np.asarray(bq)[hs] * scale).astype(np.float32)
        bk_s = np.asarray(bk)[hs].astype(np.float32)
        bv_s = (np.asarray(bv)[hs] * V_SCALE).astype(np.float32)
        in_maps.append({
            "xT": np.ascontiguousarray(x[b].T).astype(bf16),
            "wq": np.ascontiguousarray(
                np.asarray(Wq)[:, hs] * scale).astype(bf16),
            "wk": np.ascontiguousarray(np.asarray(Wk)[:, hs]).astype(bf16),
            "wv": np.ascontiguousarray(
                np.asarray(Wv)[:, hs] * V_SCALE).astype(bf16),
            "wo": np.ascontiguousarray(
                np.asarray(Wo)[hs, :] / V_SCALE).astype(bf16),
            "bq": np.ascontiguousarray(bq_s.reshape(-1, P).T),
            "bk": np.ascontiguousarray(bk_s.reshape(-1, P).T),
            "bvb": np.ascontiguousarray(
                np.broadcast_to(bv_s, (P, HD))
            ),
        })
    return in_maps


_NC_CACHE = {}


def _get_nc():
    if "nc" not in _NC_CACHE:
        _NC_CACHE["nc"] = build_nc()
    return _NC_CACHE["nc"]


def kernel(x, Wq, bq, Wk, bk, Wv, bv, Wo, bo, _trace=False, _trace_kwargs=None):
    from concourse.bass_utils import run_bass_kernel_spmd

    x = np.asarray(x, dtype=np.float32)
    B, S, E = x.shape
    nc = _get_nc()
    in_maps = make_in_maps(x, Wq, bq, Wk, bk, Wv, bv, Wo)
    res = run_bass_kernel_spmd(
        nc, in_maps, list(range(N_CORES)),
        trace=_trace, **(_trace_kwargs or {}),
    )
    groups = N_CORES // B
    full = np.zeros((B, S, E), dtype=np.float64)
    for c in range(N_CORES):
        full[c // groups] += res.results[c]["out"]
    full += np.asarray(bo, dtype=np.float64)
    out = full.astype(np.float32)
    if _trace:
        return out, res
    return out
